# revision 31
# baseline (speedup 1.0000x reference)
"""Trainium2 Bass kernel for nn_Generator_LSTM_23433341567859.

Pipeline (see reference.py):
  1. init LSTM (tanh) over [B=65536, T=32, F=16] -> final (h, c), D=8
  2. batch-norm h and c (training stats over full batch -> AllReduce)
  3. 14 autoregressive LSTM (relu) steps, input = h itself
  4. reverse the 14 outputs, batch-norm (stats over B*14 -> AllReduce)
  5. backward LSTM (relu, Do=4), return_sequences -> [B, 14, 4]

Distribution: pure data-parallel over 8 NeuronCores (8192 batch rows
per core); tiny weights replicated; the two batch-norm statistics are
combined with on-device AllReduces (sum / sum-of-squares, 32 floats).

On-chip layout (per core):
  batch 8192 = 16 groups x 512.  State tiles H, C are [128, 512] fp32:
  partition p = 8*g + d (group-major, channel minor), free j = index
  inside the group.  Gate pre-activations are produced by block-diagonal
  packed matmuls straight into per-gate PSUM banks z[:, gate*512:...]
  with the SAME [16g x 8d, 512] layout, so every activation and
  elementwise op runs at full 128-partition width with FD=512.
  x is pre-transposed on the host to [T, 2, 128, 512] per core
  (partition = 8 groups x 16 features), giving fully contiguous DMA.
"""
import os
import sys

import numpy as np

sys.path.insert(0, "/opt/trn_rl_repo")

B, T, F, D = 65536, 32, 16, 8
G, J = 16, 512          # batch groups per core x group size
ROWS, DO = 14, 4
NCORES = 8
BC = B // NCORES        # 8192 batch rows per core
EPS = 1e-3

_PROG = None            # cached (nc, input names)


# ---------------------------------------------------------------------------
# device program
# ---------------------------------------------------------------------------

def build_body(ctx, tc, a):
    """Emit the kernel IR.  `a` maps logical names -> DRAM APs:
    x [T,2,128,J], wx [4,128,64], uh [4,128,128], u12 [4,128,128],
    w2b [4,128,64], u2b [4,64,64], bx [4,128,1], b1v [4,128,1],
    b2v [4,64,1], bn [6,128,1], emat [128,8], out [ROWS,64,J].
    """
    import concourse.bass as bass
    import concourse.mybir as mybir

    nc = tc.nc
    f32 = mybir.dt.float32
    f32r = mybir.dt.float32r
    AF = mybir.ActivationFunctionType
    OP = mybir.AluOpType
    r_ = lambda ap: ap if ap.dtype == f32r else ap.bitcast(f32r)

    consts = ctx.enter_context(tc.tile_pool(name="consts", bufs=1))
    xpool = ctx.enter_context(tc.tile_pool(name="xp", bufs=3))
    state = ctx.enter_context(tc.tile_pool(name="state", bufs=1))
    apool = ctx.enter_context(tc.tile_pool(name="acts", bufs=2))
    h3pool = ctx.enter_context(tc.tile_pool(name="h3p", bufs=2))
    zpool = ctx.enter_context(tc.tile_pool(name="zp", bufs=2,
                                           space=bass.MemorySpace.PSUM))
    dpool = ctx.enter_context(tc.tile_pool(name="dp", bufs=1,
                                           space=bass.MemorySpace.DRAM))
    spool = ctx.enter_context(tc.tile_pool(name="sp", bufs=1))

    # ---- load constants -------------------------------------------------
    # Tiles that feed FP32r matmuls are declared float32r end-to-end: the
    # BIR verifier requires every producer of an f32r operand to emit
    # f32r-rounded output.
    def load(src, shape, tag, dt=f32):
        t = consts.tile(shape, dt, name=tag, tag=tag)
        nc.sync.dma_start(t[:], src.bitcast(dt) if dt is f32r else src)
        return t

    wxA = [load(a["wx"][g][0], [128, 128], f"wxA{g}", f32r) for g in range(4)]
    wxB = [load(a["wx"][g][1], [128, 128], f"wxB{g}", f32r) for g in range(4)]
    uh = [load(a["uh"][g], [128, 128], f"uh{g}", f32r) for g in range(4)]
    u12 = [load(a["u12"][g], [128, 128], f"u12_{g}", f32r) for g in range(4)]
    w2b = [load(a["w2b"][g], [128, 64], f"w2b{g}") for g in range(4)]
    u2b = [load(a["u2b"][g], [64, 64], f"u2b{g}", f32r) for g in range(4)]
    bx = [load(a["bx"][g], [128, 1], f"bx{g}") for g in range(4)]
    b1v = [load(a["b1v"][g], [128, 1], f"b1v{g}") for g in range(4)]
    b2v = [load(a["b2v"][g], [64, 1], f"b2v{g}") for g in range(4)]
    bn = [load(a["bn"][g], [128, 1], f"bn{g}") for g in range(6)]
    emat = load(a["emat"], [128, 8], "emat")

    # Two independent batch streams (free-dim halves) so the serial
    # per-step elementwise chain of one stream hides under the other
    # stream's engine work.  Matmuls and PSUM banks stay full-width
    # (concurrent PE-write + ACT-read of one PSUM bank is a HW-fatal
    # hazard, so banks are never split across asynchronous streams).
    NS, JH = 2, J // 2

    C = [state.tile([128, JH], f32, name=f"C{s}") for s in range(NS)]
    H = state.tile([128, J], f32, name="H")
    for s in range(NS):
        nc.gpsimd.memset(C[s][:], 0.0)

    def gates_elemwise(z, s, cur_c, out_h, relu_mode, biases, np_):
        """sigma/tanh the four gate banks of PSUM tile z (partitions
        [0:np_], stream s's free half) and apply the LSTM cell update in
        place on cur_c, writing the new hidden state to out_h."""
        sig = []
        for g, func in ((0, AF.Sigmoid), (1, AF.Sigmoid),
                        (2, AF.Relu if relu_mode else AF.Tanh),
                        (3, AF.Sigmoid)):
            sg = apool.tile([np_, JH], f32, name=f"sg{g}",
                            tag=f"sg{np_}_{g}_{s}")
            nc.scalar.activation(sg[:], z[0:np_, g * J + s * JH:
                                          g * J + (s + 1) * JH],
                                 func, bias=biases[g][:])
            sig.append(sg)
        t1 = apool.tile([np_, JH], f32, name="t1", tag=f"t1_{np_}_{s}")
        nc.vector.tensor_mul(t1[:], sig[0][:], sig[2][:])
        nc.vector.tensor_mul(cur_c[:], sig[1][:], cur_c[:])
        nc.vector.tensor_add(cur_c[:], cur_c[:], t1[:])
        if relu_mode:
            # h = sigmoid(o) * relu(c), fused on DVE
            nc.vector.scalar_tensor_tensor(out_h.bitcast(f32r), cur_c[:],
                                           0.0, sig[3][:], OP.max, OP.mult)
        else:
            tc_t = apool.tile([np_, JH], f32, name="tc_t", tag=f"tc{np_}_{s}")
            nc.scalar.activation(tc_t[:], cur_c[:], AF.Tanh)
            nc.vector.tensor_mul(out_h.bitcast(f32r), sig[3][:], tc_t[:])

    # A tiny warmup AllReduce issued up-front: it runs on the collective
    # cores concurrently with phase 1 and absorbs the first-collective
    # setup cost plus cross-core NEFF-start skew, so the batch-norm
    # AllReduce on the critical path hits a warmed, synchronized fabric.
    wu_in = dpool.tile([8, 4], f32, name="wuin")
    wu_out = dpool.tile([8, 4], f32, name="wuout", addr_space="Shared")
    nc.sync.dma_start(wu_in[:], emat[0:8, 0:4])
    nc.gpsimd.collective_compute(
        "AllReduce", mybir.AluOpType.add,
        replica_groups=[list(range(NCORES))],
        ins=[wu_in[:].opt()], outs=[wu_out[:].opt()])

    # ---- phase 1: init LSTM (tanh), 32 steps ----------------------------
    # Timesteps are processed in pairs: the x-part matmuls for both steps
    # are issued one pair AHEAD, grouped by weight (one LDWEIGHTS per
    # weight per pair), so the PE always has independent work while the
    # recurrent h-part of the current pair waits on the elementwise chain
    # (keeps the HAM clock-gate warm).  The x-parts open the PSUM groups;
    # the h-part matmul of each step closes its banks.
    def x_pair(tp):
        xt = xpool.tile([128, 4 * J], f32r, name="xt", tag="xt")
        nc.sync.dma_start(
            xt[:], a["x"][tp:tp + 2].rearrange("t h p j -> p t h j")
            .bitcast(f32r))
        zz = [zpool.tile([128, 4 * J], f32, name=f"z{k}", tag="z")
              for k in range(2)]
        for g in range(4):
            for k in range(2):
                nc.tensor.matmul(zz[k][:, g * J:(g + 1) * J], wxA[g][:],
                                 xt[:, (2 * k) * J:(2 * k + 1) * J],
                                 start=True, stop=False, skip_group_check=True)
        for g in range(4):
            for k in range(2):
                nc.tensor.matmul(zz[k][:, g * J:(g + 1) * J], wxB[g][:],
                                 xt[:, (2 * k + 1) * J:(2 * k + 2) * J],
                                 start=False, stop=(tp + k == 0),
                                 skip_group_check=True)
        return zz

    pend = x_pair(0)
    for tp in range(0, T, 2):
        nxt = x_pair(tp + 2) if tp + 2 < T else None
        for k in range(2):
            t = tp + k
            if t > 0:
                for g in range(4):
                    nc.tensor.matmul(pend[k][:, g * J:(g + 1) * J],
                                     uh[g][:], r_(H[:]), start=False,
                                     stop=True, skip_group_check=True)
            for s in range(NS):
                gates_elemwise(pend[k], s, C[s], H[:, s * JH:(s + 1) * JH],
                               relu_mode=False, biases=bx, np_=128)
        pend = nxt

    # ---- batch-norm stats for h and c + AllReduce -----------------------
    scr = apool.tile([128, J], f32, name="scr", tag="scr")
    st8 = spool.tile([128, 8], f32, name="st8")
    nc.scalar.activation(scr[:], H[:], AF.Identity, accum_out=st8[:, 0:1])
    nc.scalar.activation(scr[:], H[:], AF.Square, accum_out=st8[:, 1:2])
    for s in range(NS):
        nc.scalar.activation(scr[:, 0:JH], C[s][:], AF.Identity,
                             accum_out=st8[:, 2 + 2 * s:3 + 2 * s])
        nc.scalar.activation(scr[:, 0:JH], C[s][:], AF.Square,
                             accum_out=st8[:, 3 + 2 * s:4 + 2 * s])
    st4 = spool.tile([128, 4], f32, name="st4")
    nc.vector.tensor_copy(st4[:, 0:2], st8[:, 0:2])
    nc.vector.tensor_add(st4[:, 2:3], st8[:, 2:3], st8[:, 4:5])
    nc.vector.tensor_add(st4[:, 3:4], st8[:, 3:4], st8[:, 5:6])

    def allreduce_stats(st, width, tag):
        """Fold [128, width] per-partition sums to per-channel [8, width]
        and AllReduce across the 8 cores.  Returns an SBUF [8, width]."""
        zf = zpool.tile([128, 4 * J], f32, name=f"zf{tag}", tag="z")
        nc.tensor.matmul(zf[0:8, 0:width], emat[:], st[:], start=True, stop=True)
        fold = spool.tile([8, width], f32, name=f"fold{tag}", tag=f"fold{tag}")
        nc.vector.tensor_copy(fold[:], zf[0:8, 0:width])
        cc_in = dpool.tile([8, width], f32, name=f"ccin{tag}", tag=f"ccin{tag}")
        cc_out = dpool.tile([8, width], f32, name=f"ccout{tag}",
                            tag=f"ccout{tag}", addr_space="Shared")
        nc.sync.dma_start(cc_in[:], fold[:])
        nc.gpsimd.collective_compute(
            "AllReduce", mybir.AluOpType.add,
            replica_groups=[list(range(NCORES))],
            ins=[cc_in[:].opt()], outs=[cc_out[:].opt()])
        allst = spool.tile([8, width], f32, name=f"allst{tag}", tag=f"allst{tag}")
        nc.sync.dma_start(allst[:], cc_out[:])
        return allst

    def bn_coeffs(sum_ap, sq_ap, n, gamma, beta, out2):
        """out2[:, 0:1] = scale, out2[:, 1:2] = shift for
        y = x*scale + shift  (training-mode batch norm)."""
        m = spool.tile([8, 4], f32, name="bnm", tag="bnm")
        nc.vector.tensor_scalar_mul(m[:, 0:1], sum_ap, 1.0 / n)   # mean
        nc.vector.tensor_scalar_mul(m[:, 1:2], sq_ap, 1.0 / n)    # E[x^2]
        nc.vector.tensor_mul(m[:, 2:3], m[:, 0:1], m[:, 0:1])     # mean^2
        nc.vector.tensor_sub(m[:, 3:4], m[:, 1:2], m[:, 2:3])     # var
        nc.vector.tensor_scalar_add(m[:, 3:4], m[:, 3:4], EPS)
        rec = spool.tile([8, 1], f32, name="bnr", tag="bnr")
        nc.vector.reciprocal(rec[:], m[:, 3:4])
        inv = spool.tile([8, 1], f32, name="bni", tag="bni")
        nc.scalar.activation(inv[:], rec[:], AF.Sqrt)             # 1/std
        nc.vector.tensor_mul(out2[:, 0:1], inv[:], gamma)         # scale
        nc.vector.tensor_mul(rec[:], m[:, 0:1], out2[:, 0:1])     # mean*scale
        nc.vector.tensor_sub(out2[:, 1:2], beta, rec[:])          # shift

    def broadcast128(src_sb, width, tag):
        """Replicate an [8, width] SBUF tile to [128, width] via DRAM."""
        d = dpool.tile([8, width], f32, name=f"bd{tag}", tag=f"bd{tag}")
        nc.sync.dma_start(d[:], src_sb[:])
        bc = spool.tile([128, width], f32, name=f"bc{tag}", tag=f"bc{tag}")
        for k in range(16):
            nc.sync.dma_start(bc[8 * k:8 * k + 8, :], d[:])
        return bc

    allst = allreduce_stats(st4[:, 0:4], 4, "hc")
    sb4 = spool.tile([8, 4], f32, name="sb4")
    bn_coeffs(allst[:, 0:1], allst[:, 1:2], float(B), bn[0][0:8, :],
              bn[1][0:8, :], sb4[:, 0:2])
    bn_coeffs(allst[:, 2:3], allst[:, 3:4], float(B), bn[2][0:8, :],
              bn[3][0:8, :], sb4[:, 2:4])
    bc4 = broadcast128(sb4, 4, "hc")
    nc.scalar.activation(H[:].bitcast(f32r), H[:], AF.Identity,
                         bias=bc4[:, 1:2], scale=bc4[:, 0:1])
    for s in range(NS):
        nc.scalar.activation(C[s][:], C[s][:], AF.Identity,
                             bias=bc4[:, 3:4], scale=bc4[:, 2:3])

    # ---- phase 2: autoregressive LSTM (relu), 14 steps ------------------
    hs = state.tile([128, ROWS * J], f32, name="hs")
    hin = H[:]
    for r in range(ROWS):
        z = zpool.tile([128, 4 * J], f32, name="z2", tag="z")
        for g in range(4):
            nc.tensor.matmul(z[:, g * J:(g + 1) * J], r_(u12[g][:]),
                             r_(hin), start=True, stop=True)
        for s in range(NS):
            gates_elemwise(z, s, C[s],
                           hs[:, r * J + s * JH:r * J + (s + 1) * JH],
                           relu_mode=True, biases=b1v, np_=128)
        hin = hs[:, r * J:(r + 1) * J]

    # ---- batch-norm stats for bars + AllReduce --------------------------
    scr3 = state.tile([128, ROWS * J], f32, name="scr3")
    st2 = spool.tile([128, 2], f32, name="st2")
    nc.scalar.activation(scr3[:], hs[:], AF.Identity, accum_out=st2[:, 0:1])
    nc.scalar.activation(scr3[:], hs[:], AF.Square, accum_out=st2[:, 1:2])
    allst3 = allreduce_stats(st2[:, 0:2], 2, "b3")
    sb2 = spool.tile([8, 2], f32, name="sb2")
    bn_coeffs(allst3[:, 0:1], allst3[:, 1:2], float(B * ROWS),
              bn[4][0:8, :], bn[5][0:8, :], sb2[:, 0:2])
    bc2 = broadcast128(sb2, 2, "b3")

    # Fold BN-3 into the backward-LSTM input weights/biases:
    #   (x*s + m) @ W2 = x @ (diag(s) W2) + (m @ W2)
    w2s = []
    for g in range(4):
        w = spool.tile([128, 64], f32r, name=f"w2s{g}", tag=f"w2s{g}")
        nc.vector.tensor_scalar_mul(w[:], w2b[g][:], bc2[:, 0:1])
        w2s.append(w)
    zb = zpool.tile([128, 4 * J], f32, name="zb", tag="z")
    for g in range(4):
        nc.tensor.matmul(zb[0:64, g:g + 1], w2b[g][:], bc2[:, 1:2],
                         start=True, stop=True)
    b3eff = []
    for g in range(4):
        bg = spool.tile([64, 1], f32, name=f"b3e{g}", tag=f"b3e{g}")
        nc.vector.scalar_tensor_tensor(bg[:], zb[0:64, g:g + 1], 1.0,
                                       b2v[g][:], OP.mult, OP.add)
        b3eff.append(bg)

    # ---- phase 3: backward LSTM (relu), reversed inputs -----------------
    # Same software pipeline: the input-side matmuls (which depend only on
    # hs) are issued one step ahead of the recurrent h-part.
    C3 = [state.tile([64, JH], f32, name=f"C3{s}") for s in range(NS)]
    for s in range(NS):
        nc.gpsimd.memset(C3[s][:], 0.0)

    def x3_step(r):
        src = hs[:, (ROWS - 1 - r) * J:(ROWS - r) * J]
        z = zpool.tile([128, 4 * J], f32, name="z3", tag="z")
        for g in range(4):
            nc.tensor.matmul(z[0:64, g * J:(g + 1) * J], r_(w2s[g][:]),
                             r_(src), start=True, stop=(r == 0),
                             skip_group_check=True)
        return z

    h3cur = None
    zc = x3_step(0)
    for r in range(ROWS):
        zn = x3_step(r + 1) if r + 1 < ROWS else None
        if r > 0:
            for g in range(4):
                nc.tensor.matmul(zc[0:64, g * J:(g + 1) * J], r_(u2b[g][:]),
                                 r_(h3cur[:]), start=False, stop=True,
                                 skip_group_check=True)
        h3 = h3pool.tile([64, J], f32, name="h3", tag="h3")
        for s in range(NS):
            gates_elemwise(zc, s, C3[s], h3[:, s * JH:(s + 1) * JH],
                           relu_mode=True, biases=b3eff, np_=64)
        nc.sync.dma_start(a["out"][r], h3[:])
        h3cur = h3
        zc = zn


def _build_program():
    import concourse.bacc as bacc
    import concourse.tile as tile
    import concourse.mybir as mybir
    from contextlib import ExitStack

    f32 = mybir.dt.float32
    nc = bacc.Bacc("TRN2", target_bir_lowering=False, debug=False,
                   enable_asserts=True, num_devices=NCORES)
    shapes = {
        "x": [T, 2, 128, J], "wx": [4, 2, 128, 128], "uh": [4, 128, 128],
        "u12": [4, 128, 128], "w2b": [4, 128, 64], "u2b": [4, 64, 64],
        "bx": [4, 128, 1], "b1v": [4, 128, 1], "b2v": [4, 64, 1],
        "bn": [6, 128, 1], "emat": [128, 8],
    }
    aps = {}
    for name, shp in shapes.items():
        aps[name] = nc.dram_tensor(name, shp, f32, kind="ExternalInput")[:]
    aps["out"] = nc.dram_tensor("out", [ROWS, 64, J], f32,
                                kind="ExternalOutput")[:]
    with tile.TileContext(nc) as tc:
        with ExitStack() as ctx:
            build_body(ctx, tc, aps)
    nc.compile()
    return nc, list(shapes.keys())


def _get_program():
    global _PROG
    if _PROG is None:
        _PROG = _build_program()
    return _PROG


# ---------------------------------------------------------------------------
# host-side data prep
# ---------------------------------------------------------------------------

def prep_shared(W0, U0, b0, gamma_h, beta_h, gamma_c, beta_c,
                W1, U1, b1, gamma3, beta3, W2, U2, b2):
    f = np.float32
    W0, U0, b0 = np.asarray(W0, f), np.asarray(U0, f), np.asarray(b0, f)
    W1, U1, b1 = np.asarray(W1, f), np.asarray(U1, f), np.asarray(b1, f)
    W2, U2, b2 = np.asarray(W2, f), np.asarray(U2, f), np.asarray(b2, f)
    U12 = W1 + U1

    wx = np.zeros((4, 2, 128, 128), f)
    uh = np.zeros((4, 128, 128), f)
    u12 = np.zeros((4, 128, 128), f)
    w2b = np.zeros((4, 128, 64), f)
    u2b = np.zeros((4, 64, 64), f)
    for g in range(4):
        for k in range(8):
            blk = W0[:, g * D:(g + 1) * D]
            wx[g, 0, 16 * k:16 * k + 16, 8 * k:8 * k + 8] = blk
            wx[g, 1, 16 * k:16 * k + 16, 64 + 8 * k:64 + 8 * k + 8] = blk
        for k in range(G):
            uh[g, 8 * k:8 * k + 8, 8 * k:8 * k + 8] = U0[:, g * D:(g + 1) * D]
            u12[g, 8 * k:8 * k + 8, 8 * k:8 * k + 8] = U12[:, g * D:(g + 1) * D]
            w2b[g, 8 * k:8 * k + 8, 4 * k:4 * k + 4] = W2[:, g * DO:(g + 1) * DO]
            u2b[g, 4 * k:4 * k + 4, 4 * k:4 * k + 4] = U2[:, g * DO:(g + 1) * DO]

    bx = np.stack([np.tile(b0[g * D:(g + 1) * D], G) for g in range(4)])
    b1v = np.stack([np.tile(b1[g * D:(g + 1) * D], G) for g in range(4)])
    b2v = np.stack([np.tile(b2[g * DO:(g + 1) * DO], G) for g in range(4)])
    bn = np.stack([np.tile(np.asarray(v, f), G) for v in
                   (gamma_h, beta_h, gamma_c, beta_c, gamma3, beta3)])
    emat = np.zeros((128, 8), f)
    emat[np.arange(128), np.arange(128) % 8] = 1.0

    return {
        "wx": wx, "uh": uh, "u12": u12, "w2b": w2b, "u2b": u2b,
        "bx": bx[:, :, None].astype(f), "b1v": b1v[:, :, None].astype(f),
        "b2v": b2v[:, :, None].astype(f), "bn": bn[:, :, None].astype(f),
        "emat": emat,
    }


def prep_x(noise_seed):
    """[B, T, F] -> per-core [T, 2, 128, J]; partition = 8 groups x 16
    features, the two halves covering groups 0-7 / 8-15."""
    x = np.asarray(noise_seed, np.float32)
    xr = x.reshape(NCORES, G, J, T, F).transpose(0, 3, 1, 4, 2)
    return np.ascontiguousarray(xr.reshape(NCORES, T, 2, 128, J))


def assemble_out(per_core):
    """8 x [ROWS, 64, J] device tensors -> [B, ROWS, DO]."""
    dev = np.stack(per_core)                     # [8, ROWS, 64, J]
    dev = dev.reshape(NCORES, ROWS, G, DO, J).transpose(0, 2, 4, 1, 3)
    return np.ascontiguousarray(dev.reshape(B, ROWS, DO)).astype(np.float32)


def make_in_maps(noise_seed, shared):
    xs = prep_x(noise_seed)
    return [{"x": xs[c], **shared} for c in range(NCORES)]


# ---------------------------------------------------------------------------
# entry point
# ---------------------------------------------------------------------------

def kernel(noise_seed, W0, U0, b0, gamma_h, beta_h, gamma_c, beta_c,
           W1, U1, b1, gamma3, beta3, W2, U2, b2, training=None, **_):
    from concourse import bass_utils

    nc, _names = _get_program()
    shared = prep_shared(W0, U0, b0, gamma_h, beta_h, gamma_c, beta_c,
                         W1, U1, b1, gamma3, beta3, W2, U2, b2)
    in_maps = make_in_maps(noise_seed, shared)
    res = bass_utils.run_bass_kernel_spmd(
        nc, in_maps, core_ids=list(range(NCORES)),
        trace=bool(int(os.environ.get("KERNEL_TRACE", "0"))),
        tmpdir=os.environ.get("KERNEL_TRACE_DIR") or None)
    outs = [res.results[c]["out"] for c in range(NCORES)]
    out = assemble_out(outs)
    if res.exec_time_ns is not None:
        kernel.last_exec_time_ns = res.exec_time_ns
    return out


# revision 37
# speedup vs baseline: 1.1588x; 1.1588x over previous
"""Trainium2 Bass kernel for nn_Generator_LSTM_23433341567859.

Pipeline (see reference.py):
  1. init LSTM (tanh) over [B=65536, T=32, F=16] -> final (h, c), D=8
  2. batch-norm h and c (training stats over full batch -> AllReduce)
  3. 14 autoregressive LSTM (relu) steps, input = h itself
  4. reverse the 14 outputs, batch-norm (stats over B*14 -> AllReduce)
  5. backward LSTM (relu, Do=4), return_sequences -> [B, 14, 4]

Distribution: pure data-parallel over 8 NeuronCores (8192 batch rows
per core); tiny weights replicated; the two batch-norm statistics are
combined with on-device AllReduces (sum / sum-of-squares, 32 floats).

On-chip layout (per core):
  batch 8192 = 16 groups x 512.  State tiles H, C are [128, 512] fp32:
  partition p = 8*g + d (group-major, channel minor), free j = index
  inside the group.  Gate pre-activations are produced by block-diagonal
  packed matmuls straight into per-gate PSUM banks z[:, gate*512:...]
  with the SAME [16g x 8d, 512] layout, so every activation and
  elementwise op runs at full 128-partition width with FD=512.
  x is pre-transposed on the host to [T, 2, 128, 512] per core
  (partition = 8 groups x 16 features), giving fully contiguous DMA.
"""
import os
import sys

import numpy as np

sys.path.insert(0, "/opt/trn_rl_repo")

B, T, F, D = 65536, 32, 16, 8
G, J = 16, 512          # batch groups per core x group size
ROWS, DO = 14, 4
NCORES = 8
BC = B // NCORES        # 8192 batch rows per core
EPS = 1e-3

_PROG = None            # cached (nc, input names)


# ---------------------------------------------------------------------------
# device program
# ---------------------------------------------------------------------------

def build_body(ctx, tc, a):
    """Emit the kernel IR.  `a` maps logical names -> DRAM APs:
    x [T,2,128,J], wx [4,128,64], uh [4,128,128], u12 [4,128,128],
    w2b [4,128,64], u2b [4,64,64], bx [4,128,1], b1v [4,128,1],
    b2v [4,64,1], bn [6,128,1], emat [128,8], out [ROWS,64,J].
    """
    import concourse.bass as bass
    import concourse.mybir as mybir

    nc = tc.nc
    f32 = mybir.dt.float32
    f32r = mybir.dt.float32r
    AF = mybir.ActivationFunctionType
    OP = mybir.AluOpType
    r_ = lambda ap: ap if ap.dtype == f32r else ap.bitcast(f32r)

    consts = ctx.enter_context(tc.tile_pool(name="consts", bufs=1))
    xpool = ctx.enter_context(tc.tile_pool(name="xp", bufs=3))
    state = ctx.enter_context(tc.tile_pool(name="state", bufs=1))
    apool = ctx.enter_context(tc.tile_pool(name="acts", bufs=2))
    h3pool = ctx.enter_context(tc.tile_pool(name="h3p", bufs=2))
    zpool = ctx.enter_context(tc.tile_pool(name="zp", bufs=2,
                                           space=bass.MemorySpace.PSUM))
    dpool = ctx.enter_context(tc.tile_pool(name="dp", bufs=1,
                                           space=bass.MemorySpace.DRAM))
    spool = ctx.enter_context(tc.tile_pool(name="sp", bufs=1))

    # ---- load constants -------------------------------------------------
    # Tiles that feed FP32r matmuls are declared float32r end-to-end: the
    # BIR verifier requires every producer of an f32r operand to emit
    # f32r-rounded output.
    def load(src, shape, tag, dt=f32):
        t = consts.tile(shape, dt, name=tag, tag=tag)
        nc.sync.dma_start(t[:], src.bitcast(dt) if dt is f32r else src)
        return t

    wxA = [load(a["wx"][g][0], [128, 128], f"wxA{g}", f32r) for g in range(4)]
    wxB = [load(a["wx"][g][1], [128, 128], f"wxB{g}", f32r) for g in range(4)]
    uh = [load(a["uh"][g], [128, 128], f"uh{g}", f32r) for g in range(4)]
    u12 = [load(a["u12"][g], [128, 128], f"u12_{g}", f32r) for g in range(4)]
    w2b = [load(a["w2b"][g], [128, 64], f"w2b{g}") for g in range(4)]
    u2b = [load(a["u2b"][g], [64, 64], f"u2b{g}", f32r) for g in range(4)]
    bx = [load(a["bx"][g], [128, 1], f"bx{g}") for g in range(4)]
    b1v = [load(a["b1v"][g], [128, 1], f"b1v{g}") for g in range(4)]
    b2v = [load(a["b2v"][g], [64, 1], f"b2v{g}") for g in range(4)]
    bn = [load(a["bn"][g], [128, 1], f"bn{g}") for g in range(6)]
    emat = load(a["emat"], [128, 8], "emat")

    C = state.tile([128, J], f32, name="C")
    H = state.tile([128, J], f32, name="H")
    nc.gpsimd.memset(C[:], 0.0)

    def gates_elemwise(z, cur_c, out_h, relu_mode, biases, np_):
        """sigma/tanh the four gate banks of PSUM tile z (partitions
        [0:np_]) and apply the LSTM cell update in place on cur_c,
        writing the new hidden state to out_h."""
        sig = []
        for g, func in ((0, AF.Sigmoid), (1, AF.Sigmoid),
                        (2, AF.Relu if relu_mode else AF.Tanh),
                        (3, AF.Sigmoid)):
            sg = apool.tile([np_, J], f32, name=f"sg{g}", tag=f"sg{np_}_{g}")
            nc.scalar.activation(sg[:], z[0:np_, g * J:(g + 1) * J],
                                 func, bias=biases[g][:])
            sig.append(sg)
        t1 = apool.tile([np_, J], f32, name="t1", tag=f"t1_{np_}")
        nc.vector.tensor_mul(t1[:], sig[0][:], sig[2][:])
        nc.vector.tensor_mul(cur_c[:], sig[1][:], cur_c[:])
        nc.vector.tensor_add(cur_c[:], cur_c[:], t1[:])
        if relu_mode:
            # h = sigmoid(o) * relu(c), fused on DVE
            nc.vector.scalar_tensor_tensor(out_h.bitcast(f32r), cur_c[:],
                                           0.0, sig[3][:], OP.max, OP.mult)
        else:
            tc_t = apool.tile([np_, J], f32, name="tc_t", tag=f"tc{np_}")
            nc.scalar.activation(tc_t[:], cur_c[:], AF.Tanh)
            nc.vector.tensor_mul(out_h.bitcast(f32r), sig[3][:], tc_t[:])

    # A tiny warmup AllReduce issued up-front: it runs on the collective
    # cores concurrently with phase 1 and absorbs the first-collective
    # setup cost plus cross-core NEFF-start skew, so the batch-norm
    # AllReduce on the critical path hits a warmed, synchronized fabric.
    wu_in = dpool.tile([8, 4], f32, name="wuin")
    wu_out = dpool.tile([8, 4], f32, name="wuout", addr_space="Shared")
    nc.sync.dma_start(wu_in[:], emat[0:8, 0:4])
    nc.gpsimd.collective_compute(
        "AllReduce", mybir.AluOpType.add,
        replica_groups=[list(range(NCORES))],
        ins=[wu_in[:].opt()], outs=[wu_out[:].opt()])

    # ---- phase 1: init LSTM (tanh), 32 steps ----------------------------
    # Timesteps are processed in pairs: the x-part matmuls for both steps
    # are issued one pair AHEAD, grouped by weight (one LDWEIGHTS per
    # weight per pair), so the PE always has independent work while the
    # recurrent h-part of the current pair waits on the elementwise chain
    # (keeps the HAM clock-gate warm).  The x-parts open the PSUM groups;
    # the h-part matmul of each step closes its banks.
    def x_pair(tp):
        xt = xpool.tile([128, 4 * J], f32r, name="xt", tag="xt")
        nc.sync.dma_start(
            xt[:], a["x"][tp:tp + 2].rearrange("t h p j -> p t h j")
            .bitcast(f32r))
        zz = [zpool.tile([128, 4 * J], f32, name=f"z{k}", tag="z")
              for k in range(2)]
        for g in range(4):
            for k in range(2):
                nc.tensor.matmul(zz[k][:, g * J:(g + 1) * J], wxA[g][:],
                                 xt[:, (2 * k) * J:(2 * k + 1) * J],
                                 start=True, stop=False, skip_group_check=True)
        for g in range(4):
            for k in range(2):
                nc.tensor.matmul(zz[k][:, g * J:(g + 1) * J], wxB[g][:],
                                 xt[:, (2 * k + 1) * J:(2 * k + 2) * J],
                                 start=False, stop=(tp + k == 0),
                                 skip_group_check=True)
        return zz

    pend = x_pair(0)
    for tp in range(0, T, 2):
        nxt = x_pair(tp + 2) if tp + 2 < T else None
        for k in range(2):
            t = tp + k
            if t > 0:
                for g in range(4):
                    nc.tensor.matmul(pend[k][:, g * J:(g + 1) * J],
                                     uh[g][:], r_(H[:]), start=False,
                                     stop=True, skip_group_check=True)
            gates_elemwise(pend[k], C, H[:], relu_mode=False, biases=bx,
                           np_=128)
        pend = nxt

    # ---- batch-norm stats for h and c + AllReduce -----------------------
    scr = apool.tile([128, J], f32, name="scr", tag="scr")
    st4 = spool.tile([128, 4], f32, name="st4")
    nc.scalar.activation(scr[:], H[:], AF.Identity, accum_out=st4[:, 0:1])
    nc.scalar.activation(scr[:], H[:], AF.Square, accum_out=st4[:, 1:2])
    nc.scalar.activation(scr[:], C[:], AF.Identity, accum_out=st4[:, 2:3])
    nc.scalar.activation(scr[:], C[:], AF.Square, accum_out=st4[:, 3:4])

    def allreduce_stats(st, width, tag):
        """Fold [128, width] per-partition sums to per-channel [8, width]
        and AllReduce across the 8 cores.  Returns an SBUF [8, width]."""
        zf = zpool.tile([128, 4 * J], f32, name=f"zf{tag}", tag="z")
        nc.tensor.matmul(zf[0:8, 0:width], emat[:], st[:], start=True, stop=True)
        fold = spool.tile([8, width], f32, name=f"fold{tag}", tag=f"fold{tag}")
        nc.vector.tensor_copy(fold[:], zf[0:8, 0:width])
        cc_in = dpool.tile([8, width], f32, name=f"ccin{tag}", tag=f"ccin{tag}")
        cc_out = dpool.tile([8, width], f32, name=f"ccout{tag}",
                            tag=f"ccout{tag}", addr_space="Shared")
        nc.sync.dma_start(cc_in[:], fold[:])
        nc.gpsimd.collective_compute(
            "AllReduce", mybir.AluOpType.add,
            replica_groups=[list(range(NCORES))],
            ins=[cc_in[:].opt()], outs=[cc_out[:].opt()])
        allst = spool.tile([8, width], f32, name=f"allst{tag}", tag=f"allst{tag}")
        nc.sync.dma_start(allst[:], cc_out[:])
        return allst

    def bn_coeffs(sum_ap, sq_ap, n, gamma, beta, out2):
        """out2[:, 0:1] = scale, out2[:, 1:2] = shift for
        y = x*scale + shift  (training-mode batch norm)."""
        m = spool.tile([8, 4], f32, name="bnm", tag="bnm")
        nc.vector.tensor_scalar_mul(m[:, 0:1], sum_ap, 1.0 / n)   # mean
        nc.vector.tensor_scalar_mul(m[:, 1:2], sq_ap, 1.0 / n)    # E[x^2]
        nc.vector.tensor_mul(m[:, 2:3], m[:, 0:1], m[:, 0:1])     # mean^2
        nc.vector.tensor_sub(m[:, 3:4], m[:, 1:2], m[:, 2:3])     # var
        nc.vector.tensor_scalar_add(m[:, 3:4], m[:, 3:4], EPS)
        rec = spool.tile([8, 1], f32, name="bnr", tag="bnr")
        nc.vector.reciprocal(rec[:], m[:, 3:4])
        inv = spool.tile([8, 1], f32, name="bni", tag="bni")
        nc.scalar.activation(inv[:], rec[:], AF.Sqrt)             # 1/std
        nc.vector.tensor_mul(out2[:, 0:1], inv[:], gamma)         # scale
        nc.vector.tensor_mul(rec[:], m[:, 0:1], out2[:, 0:1])     # mean*scale
        nc.vector.tensor_sub(out2[:, 1:2], beta, rec[:])          # shift

    def broadcast128(src_sb, width, tag):
        """Replicate an [8, width] SBUF tile to [128, width] via DRAM."""
        d = dpool.tile([8, width], f32, name=f"bd{tag}", tag=f"bd{tag}")
        nc.sync.dma_start(d[:], src_sb[:])
        bc = spool.tile([128, width], f32, name=f"bc{tag}", tag=f"bc{tag}")
        for k in range(16):
            nc.sync.dma_start(bc[8 * k:8 * k + 8, :], d[:])
        return bc

    allst = allreduce_stats(st4[:, 0:4], 4, "hc")
    sb4 = spool.tile([8, 4], f32, name="sb4")
    bn_coeffs(allst[:, 0:1], allst[:, 1:2], float(B), bn[0][0:8, :],
              bn[1][0:8, :], sb4[:, 0:2])
    bn_coeffs(allst[:, 2:3], allst[:, 3:4], float(B), bn[2][0:8, :],
              bn[3][0:8, :], sb4[:, 2:4])
    bc4 = broadcast128(sb4, 4, "hc")
    nc.scalar.activation(H[:].bitcast(f32r), H[:], AF.Identity,
                         bias=bc4[:, 1:2], scale=bc4[:, 0:1])
    nc.scalar.activation(C[:], C[:], AF.Identity,
                         bias=bc4[:, 3:4], scale=bc4[:, 2:3])

    # ---- phase 2: autoregressive LSTM (relu), 14 steps ------------------
    hs = state.tile([128, ROWS * J], f32, name="hs")
    hin = H[:]
    for r in range(ROWS):
        z = zpool.tile([128, 4 * J], f32, name="z2", tag="z")
        for g in range(4):
            nc.tensor.matmul(z[:, g * J:(g + 1) * J], r_(u12[g][:]),
                             r_(hin), start=True, stop=True)
        gates_elemwise(z, C, hs[:, r * J:(r + 1) * J], relu_mode=True,
                       biases=b1v, np_=128)
        hin = hs[:, r * J:(r + 1) * J]

    # ---- batch-norm stats for bars + AllReduce --------------------------
    scr3 = state.tile([128, ROWS * J], f32, name="scr3")
    st2 = spool.tile([128, 2], f32, name="st2")
    nc.scalar.activation(scr3[:], hs[:], AF.Identity, accum_out=st2[:, 0:1])
    nc.scalar.activation(scr3[:], hs[:], AF.Square, accum_out=st2[:, 1:2])
    allst3 = allreduce_stats(st2[:, 0:2], 2, "b3")
    sb2 = spool.tile([8, 2], f32, name="sb2")
    bn_coeffs(allst3[:, 0:1], allst3[:, 1:2], float(B * ROWS),
              bn[4][0:8, :], bn[5][0:8, :], sb2[:, 0:2])
    bc2 = broadcast128(sb2, 2, "b3")

    # Fold BN-3 into the backward-LSTM input weights/biases:
    #   (x*s + m) @ W2 = x @ (diag(s) W2) + (m @ W2)
    w2s = []
    for g in range(4):
        w = spool.tile([128, 64], f32r, name=f"w2s{g}", tag=f"w2s{g}")
        nc.vector.tensor_scalar_mul(w[:], w2b[g][:], bc2[:, 0:1])
        w2s.append(w)
    zb = zpool.tile([128, 4 * J], f32, name="zb", tag="z")
    for g in range(4):
        nc.tensor.matmul(zb[0:64, g:g + 1], w2b[g][:], bc2[:, 1:2],
                         start=True, stop=True)
    b3eff = []
    for g in range(4):
        bg = spool.tile([64, 1], f32, name=f"b3e{g}", tag=f"b3e{g}")
        nc.vector.scalar_tensor_tensor(bg[:], zb[0:64, g:g + 1], 1.0,
                                       b2v[g][:], OP.mult, OP.add)
        b3eff.append(bg)

    # ---- phase 3: backward LSTM (relu), reversed inputs -----------------
    # Same software pipeline: the input-side matmuls (which depend only on
    # hs) are issued one step ahead of the recurrent h-part.
    C3 = state.tile([64, J], f32, name="C3")
    nc.gpsimd.memset(C3[:], 0.0)

    def x3_step(r):
        src = hs[:, (ROWS - 1 - r) * J:(ROWS - r) * J]
        z = zpool.tile([128, 4 * J], f32, name="z3", tag="z")
        for g in range(4):
            nc.tensor.matmul(z[0:64, g * J:(g + 1) * J], r_(w2s[g][:]),
                             r_(src), start=True, stop=(r == 0),
                             skip_group_check=True)
        return z

    h3cur = None
    zc = x3_step(0)
    for r in range(ROWS):
        zn = x3_step(r + 1) if r + 1 < ROWS else None
        if r > 0:
            for g in range(4):
                nc.tensor.matmul(zc[0:64, g * J:(g + 1) * J], r_(u2b[g][:]),
                                 r_(h3cur[:]), start=False, stop=True,
                                 skip_group_check=True)
        h3 = h3pool.tile([64, J], f32, name="h3", tag="h3")
        gates_elemwise(zc, C3, h3[:], relu_mode=True, biases=b3eff, np_=64)
        nc.sync.dma_start(a["out"][r], h3[:])
        h3cur = h3
        zc = zn


def _build_program():
    import concourse.bacc as bacc
    import concourse.tile as tile
    import concourse.mybir as mybir
    from contextlib import ExitStack

    f32 = mybir.dt.float32
    nc = bacc.Bacc("TRN2", target_bir_lowering=False, debug=False,
                   enable_asserts=True, num_devices=NCORES)
    shapes = {
        "x": [T, 2, 128, J], "wx": [4, 2, 128, 128], "uh": [4, 128, 128],
        "u12": [4, 128, 128], "w2b": [4, 128, 64], "u2b": [4, 64, 64],
        "bx": [4, 128, 1], "b1v": [4, 128, 1], "b2v": [4, 64, 1],
        "bn": [6, 128, 1], "emat": [128, 8],
    }
    aps = {}
    for name, shp in shapes.items():
        aps[name] = nc.dram_tensor(name, shp, f32, kind="ExternalInput")[:]
    aps["out"] = nc.dram_tensor("out", [ROWS, 64, J], f32,
                                kind="ExternalOutput")[:]
    with tile.TileContext(nc) as tc:
        with ExitStack() as ctx:
            build_body(ctx, tc, aps)
    nc.compile()
    return nc, list(shapes.keys())


def _get_program():
    global _PROG
    if _PROG is None:
        _PROG = _build_program()
    return _PROG


# ---------------------------------------------------------------------------
# host-side data prep
# ---------------------------------------------------------------------------

def prep_shared(W0, U0, b0, gamma_h, beta_h, gamma_c, beta_c,
                W1, U1, b1, gamma3, beta3, W2, U2, b2):
    f = np.float32
    W0, U0, b0 = np.asarray(W0, f), np.asarray(U0, f), np.asarray(b0, f)
    W1, U1, b1 = np.asarray(W1, f), np.asarray(U1, f), np.asarray(b1, f)
    W2, U2, b2 = np.asarray(W2, f), np.asarray(U2, f), np.asarray(b2, f)
    U12 = W1 + U1

    wx = np.zeros((4, 2, 128, 128), f)
    uh = np.zeros((4, 128, 128), f)
    u12 = np.zeros((4, 128, 128), f)
    w2b = np.zeros((4, 128, 64), f)
    u2b = np.zeros((4, 64, 64), f)
    for g in range(4):
        for k in range(8):
            blk = W0[:, g * D:(g + 1) * D]
            wx[g, 0, 16 * k:16 * k + 16, 8 * k:8 * k + 8] = blk
            wx[g, 1, 16 * k:16 * k + 16, 64 + 8 * k:64 + 8 * k + 8] = blk
        for k in range(G):
            uh[g, 8 * k:8 * k + 8, 8 * k:8 * k + 8] = U0[:, g * D:(g + 1) * D]
            u12[g, 8 * k:8 * k + 8, 8 * k:8 * k + 8] = U12[:, g * D:(g + 1) * D]
            w2b[g, 8 * k:8 * k + 8, 4 * k:4 * k + 4] = W2[:, g * DO:(g + 1) * DO]
            u2b[g, 4 * k:4 * k + 4, 4 * k:4 * k + 4] = U2[:, g * DO:(g + 1) * DO]

    bx = np.stack([np.tile(b0[g * D:(g + 1) * D], G) for g in range(4)])
    b1v = np.stack([np.tile(b1[g * D:(g + 1) * D], G) for g in range(4)])
    b2v = np.stack([np.tile(b2[g * DO:(g + 1) * DO], G) for g in range(4)])
    bn = np.stack([np.tile(np.asarray(v, f), G) for v in
                   (gamma_h, beta_h, gamma_c, beta_c, gamma3, beta3)])
    emat = np.zeros((128, 8), f)
    emat[np.arange(128), np.arange(128) % 8] = 1.0

    return {
        "wx": wx, "uh": uh, "u12": u12, "w2b": w2b, "u2b": u2b,
        "bx": bx[:, :, None].astype(f), "b1v": b1v[:, :, None].astype(f),
        "b2v": b2v[:, :, None].astype(f), "bn": bn[:, :, None].astype(f),
        "emat": emat,
    }


def prep_x(noise_seed):
    """[B, T, F] -> per-core [T, 2, 128, J]; partition = 8 groups x 16
    features, the two halves covering groups 0-7 / 8-15."""
    x = np.asarray(noise_seed, np.float32)
    xr = x.reshape(NCORES, G, J, T, F).transpose(0, 3, 1, 4, 2)
    return np.ascontiguousarray(xr.reshape(NCORES, T, 2, 128, J))


def assemble_out(per_core):
    """8 x [ROWS, 64, J] device tensors -> [B, ROWS, DO]."""
    dev = np.stack(per_core)                     # [8, ROWS, 64, J]
    dev = dev.reshape(NCORES, ROWS, G, DO, J).transpose(0, 2, 4, 1, 3)
    return np.ascontiguousarray(dev.reshape(B, ROWS, DO)).astype(np.float32)


def make_in_maps(noise_seed, shared):
    xs = prep_x(noise_seed)
    return [{"x": xs[c], **shared} for c in range(NCORES)]


# ---------------------------------------------------------------------------
# entry point
# ---------------------------------------------------------------------------

def kernel(noise_seed, W0, U0, b0, gamma_h, beta_h, gamma_c, beta_c,
           W1, U1, b1, gamma3, beta3, W2, U2, b2, training=None, **_):
    from concourse import bass_utils

    nc, _names = _get_program()
    shared = prep_shared(W0, U0, b0, gamma_h, beta_h, gamma_c, beta_c,
                         W1, U1, b1, gamma3, beta3, W2, U2, b2)
    in_maps = make_in_maps(noise_seed, shared)
    res = bass_utils.run_bass_kernel_spmd(
        nc, in_maps, core_ids=list(range(NCORES)),
        trace=bool(int(os.environ.get("KERNEL_TRACE", "0"))),
        tmpdir=os.environ.get("KERNEL_TRACE_DIR") or None)
    outs = [res.results[c]["out"] for c in range(NCORES)]
    out = assemble_out(outs)
    if res.exec_time_ns is not None:
        kernel.last_exec_time_ns = res.exec_time_ns
    return out


# revision 52
# speedup vs baseline: 1.1814x; 1.0195x over previous
"""Trainium2 Bass kernel for nn_Generator_LSTM_23433341567859.

Pipeline (see reference.py):
  1. init LSTM (tanh) over [B=65536, T=32, F=16] -> final (h, c), D=8
  2. batch-norm h and c (training stats over full batch -> AllReduce)
  3. 14 autoregressive LSTM (relu) steps, input = h itself
  4. reverse the 14 outputs, batch-norm (stats over B*14 -> AllReduce)
  5. backward LSTM (relu, Do=4), return_sequences -> [B, 14, 4]

Distribution: pure data-parallel over 8 NeuronCores (8192 batch rows
per core); tiny weights replicated; the two batch-norm statistics are
combined with on-device AllReduces (sum / sum-of-squares, 32 floats).

On-chip layout (per core):
  batch 8192 = 16 groups x 512.  State tiles H, C are [128, 512] fp32:
  partition p = 8*g + d (group-major, channel minor), free j = index
  inside the group.  Gate pre-activations are produced by block-diagonal
  packed matmuls straight into per-gate PSUM banks z[:, gate*512:...]
  with the SAME [16g x 8d, 512] layout, so every activation and
  elementwise op runs at full 128-partition width with FD=512.
  x is pre-transposed on the host to [T, 2, 128, 512] per core
  (partition = 8 groups x 16 features), giving fully contiguous DMA.
"""
import dataclasses
import os
import sys

import numpy as np

sys.path.insert(0, "/opt/trn_rl_repo")

B, T, F, D = 65536, 32, 16, 8
G, J = 16, 512          # batch groups per core x group size
ROWS, DO = 14, 4
NCORES = 8
BC = B // NCORES        # 8192 batch rows per core
EPS = 1e-3

_PROG = None            # cached (nc, input names)


# ---------------------------------------------------------------------------
# device program
# ---------------------------------------------------------------------------

def build_body(ctx, tc, a, zero_bias=True):
    """Emit the kernel IR.  `a` maps logical names -> DRAM APs:
    x [T,2,128,J], wx [4,128,64], uh [4,128,128], u12 [4,128,128],
    w2b [4,128,64], u2b [4,64,64], bx [4,128,1], b1v [4,128,1],
    b2v [4,64,1], bn [6,128,1], emat [128,8], out [ROWS,64,J].
    """
    import concourse.bass as bass
    import concourse.mybir as mybir

    nc = tc.nc
    f32 = mybir.dt.float32
    f32r = mybir.dt.float32r
    AF = mybir.ActivationFunctionType
    OP = mybir.AluOpType
    r_ = lambda ap: ap if ap.dtype == f32r else ap.bitcast(f32r)

    consts = ctx.enter_context(tc.tile_pool(name="consts", bufs=1))
    xpool = ctx.enter_context(tc.tile_pool(name="xp", bufs=3))
    state = ctx.enter_context(tc.tile_pool(name="state", bufs=1))
    apool = ctx.enter_context(tc.tile_pool(name="acts", bufs=2))
    h3pool = ctx.enter_context(tc.tile_pool(name="h3p", bufs=2))
    zpool = ctx.enter_context(tc.tile_pool(name="zp", bufs=2,
                                           space=bass.MemorySpace.PSUM))
    dpool = ctx.enter_context(tc.tile_pool(name="dp", bufs=1,
                                           space=bass.MemorySpace.DRAM))
    spool = ctx.enter_context(tc.tile_pool(name="sp", bufs=1))

    # ---- load constants -------------------------------------------------
    # Tiles that feed FP32r matmuls are declared float32r end-to-end: the
    # BIR verifier requires every producer of an f32r operand to emit
    # f32r-rounded output.
    def load(src, shape, tag, dt=f32):
        t = consts.tile(shape, dt, name=tag, tag=tag)
        nc.sync.dma_start(t[:], src.bitcast(dt) if dt is f32r else src)
        return t

    wxA = [load(a["wx"][g][0], [128, 128], f"wxA{g}", f32r) for g in range(4)]
    wxB = [load(a["wx"][g][1], [128, 128], f"wxB{g}", f32r) for g in range(4)]
    uh = [load(a["uh"][g], [128, 128], f"uh{g}", f32r) for g in range(4)]
    u12 = [load(a["u12"][g], [128, 128], f"u12_{g}", f32r) for g in range(4)]
    w2b = [load(a["w2b"][g], [128, 64], f"w2b{g}") for g in range(4)]
    u2b = [load(a["u2b"][g], [64, 64], f"u2b{g}", f32r) for g in range(4)]
    bx = [load(a["bx"][g], [128, 1], f"bx{g}") for g in range(4)]
    b1v = [load(a["b1v"][g], [128, 1], f"b1v{g}") for g in range(4)]
    b2v = [load(a["b2v"][g], [64, 1], f"b2v{g}") for g in range(4)]
    bn6 = load(a["bn6"], [8, 6], "bn6")
    emat = load(a["emat"], [128, 8], "emat")
    magic = consts.tile([8, 2], mybir.dt.int32, name="magic", tag="magic")
    nc.sync.dma_start(magic[:], a["magic"])

    C = state.tile([128, J], f32, name="C")
    H = state.tile([128, J], f32, name="H")
    nc.gpsimd.memset(C[:], 0.0)

    def gates_elemwise(z, cur_c, out_h, relu_mode, biases, np_,
                       relu_bias=None):
        """sigma/tanh the four gate banks of PSUM tile z (partitions
        [0:np_]) and apply the LSTM cell update in place on cur_c,
        writing the new hidden state to out_h.

        Activation/matmul issue order is g-gate first so the t1 chain can
        start as early as possible.  In relu mode the g-gate relu is fused
        into DVE ops reading the PSUM bank directly (t1 = sigmoid(i) *
        relu(z_g [+ relu_bias])) instead of a ScalarE pass.
        """
        def bank(g):
            return z[0:np_, g * J:(g + 1) * J]

        def act(g, func):
            sg = apool.tile([np_, J], f32, name=f"sg{g}", tag=f"sg{np_}_{g}")
            b = 0.0 if biases is None else biases[g][:]
            nc.scalar.activation(sg[:], bank(g), func, bias=b)
            return sg

        t1 = apool.tile([np_, J], f32, name="t1", tag=f"t1_{np_}")
        if relu_mode:
            sig_i = act(0, AF.Sigmoid)
            if relu_bias is not None:
                rg = apool.tile([np_, J], f32, name="rg", tag=f"rg_{np_}")
                nc.vector.tensor_scalar(rg[:], bank(2), relu_bias[:], 0.0,
                                        OP.add, OP.max)
                nc.vector.tensor_mul(t1[:], sig_i[:], rg[:])
            else:
                nc.vector.scalar_tensor_tensor(t1[:], bank(2), 0.0,
                                               sig_i[:], OP.max, OP.mult)
            sig_f = act(1, AF.Sigmoid)
            sig_o = act(3, AF.Sigmoid)
            nc.vector.tensor_mul(cur_c[:], sig_f[:], cur_c[:])
            nc.vector.tensor_add(cur_c[:], cur_c[:], t1[:])
            # h = sigmoid(o) * relu(c), fused on DVE
            nc.vector.scalar_tensor_tensor(out_h.bitcast(f32r), cur_c[:],
                                           0.0, sig_o[:], OP.max, OP.mult)
        else:
            tau_g = act(2, AF.Tanh)
            sig_i = act(0, AF.Sigmoid)
            sig_f = act(1, AF.Sigmoid)
            sig_o = act(3, AF.Sigmoid)
            nc.vector.tensor_mul(t1[:], sig_i[:], tau_g[:])
            nc.vector.tensor_mul(cur_c[:], sig_f[:], cur_c[:])
            nc.vector.tensor_add(cur_c[:], cur_c[:], t1[:])
            tc_t = apool.tile([np_, J], f32, name="tc_t", tag=f"tc{np_}")
            nc.scalar.activation(tc_t[:], cur_c[:], AF.Tanh)
            nc.vector.tensor_mul(out_h.bitcast(f32r), sig_o[:], tc_t[:])

    # A tiny warmup AllReduce issued up-front: it runs on the collective
    # cores concurrently with phase 1 and absorbs the first-collective
    # setup cost plus cross-core NEFF-start skew, so the batch-norm
    # AllReduce on the critical path hits a warmed, synchronized fabric.
    wu_in = dpool.tile([8, 4], f32, name="wuin")
    wu_out = dpool.tile([8, 4], f32, name="wuout", addr_space="Shared")
    nc.sync.dma_start(wu_in[:], emat[0:8, 0:4])
    nc.gpsimd.collective_compute(
        "AllReduce", mybir.AluOpType.add,
        replica_groups=[list(range(NCORES))],
        ins=[wu_in[:].opt()], outs=[wu_out[:].opt()])

    # ---- phase 1: init LSTM (tanh), 32 steps ----------------------------
    # Timesteps are processed in pairs: the x-part matmuls for both steps
    # are issued one pair AHEAD, grouped by weight (one LDWEIGHTS per
    # weight per pair), so the PE always has independent work while the
    # recurrent h-part of the current pair waits on the elementwise chain
    # (keeps the HAM clock-gate warm).  The x-parts open the PSUM groups;
    # the h-part matmul of each step closes its banks.
    def x_pair(tp):
        xt = xpool.tile([128, 4 * J], f32r, name="xt", tag="xt")
        nc.sync.dma_start(
            xt[:], a["x"][tp:tp + 2].rearrange("t h p j -> p t h j")
            .bitcast(f32r))
        zz = [zpool.tile([128, 4 * J], f32, name=f"z{k}", tag="z")
              for k in range(2)]
        for g in range(4):
            for k in range(2):
                nc.tensor.matmul(zz[k][:, g * J:(g + 1) * J], wxA[g][:],
                                 xt[:, (2 * k) * J:(2 * k + 1) * J],
                                 start=True, stop=False, skip_group_check=True)
        for g in range(4):
            for k in range(2):
                nc.tensor.matmul(zz[k][:, g * J:(g + 1) * J], wxB[g][:],
                                 xt[:, (2 * k + 1) * J:(2 * k + 2) * J],
                                 start=False, stop=(tp + k == 0),
                                 skip_group_check=True)
        return zz

    pend = x_pair(0)
    for tp in range(0, T, 2):
        nxt = x_pair(tp + 2) if tp + 2 < T else None
        for k in range(2):
            t = tp + k
            if t > 0:
                for g in (2, 0, 1, 3):   # g-gate bank first: t1 chain
                    nc.tensor.matmul(pend[k][:, g * J:(g + 1) * J],
                                     uh[g][:], r_(H[:]), start=False,
                                     stop=True, skip_group_check=True)
            gates_elemwise(pend[k], C, H[:], relu_mode=False,
                           biases=None if zero_bias else bx, np_=128)
        pend = nxt

    # ---- batch-norm stats for h and c + AllReduce -----------------------
    scr = apool.tile([128, J], f32, name="scr", tag="scr")
    st4 = spool.tile([128, 4], f32, name="st4")
    nc.scalar.activation(scr[:], H[:], AF.Identity, accum_out=st4[:, 0:1])
    nc.scalar.activation(scr[:], C[:], AF.Identity, accum_out=st4[:, 1:2])
    nc.scalar.activation(scr[:], H[:], AF.Square, accum_out=st4[:, 2:3])
    nc.scalar.activation(scr[:], C[:], AF.Square, accum_out=st4[:, 3:4])

    def allreduce_stats(st, width, tag):
        """Fold [128, width] per-partition sums to per-channel [8, width]
        and AllReduce across the 8 cores.  Returns an SBUF [8, width]."""
        zf = zpool.tile([128, 4 * J], f32, name=f"zf{tag}", tag="z")
        nc.tensor.matmul(zf[0:8, 0:width], emat[:], st[:], start=True, stop=True)
        fold = spool.tile([8, width], f32, name=f"fold{tag}", tag=f"fold{tag}")
        nc.vector.tensor_copy(fold[:], zf[0:8, 0:width])
        cc_in = dpool.tile([8, width], f32, name=f"ccin{tag}", tag=f"ccin{tag}")
        cc_out = dpool.tile([8, width], f32, name=f"ccout{tag}",
                            tag=f"ccout{tag}", addr_space="Shared")
        nc.sync.dma_start(cc_in[:], fold[:])
        nc.gpsimd.collective_compute(
            "AllReduce", mybir.AluOpType.add,
            replica_groups=[list(range(NCORES))],
            ins=[cc_in[:].opt()], outs=[cc_out[:].opt()])
        allst = spool.tile([8, width], f32, name=f"allst{tag}", tag=f"allst{tag}")
        nc.sync.dma_start(allst[:], cc_out[:])
        return allst

    i32 = mybir.dt.int32

    def bn_coeffs(sums, sqs, n, gammas, betas, scales, shifts, w, tag):
        """Vectorized over w channels-columns: scales = gamma/std,
        shifts = beta - mean*scales.  1/sqrt(var+eps) is computed on the
        DVE with the magic-constant seed plus two Newton steps, avoiding
        two ScalarE activation-table switches per sync point."""
        mean = spool.tile([8, w], f32, name="bnmean", tag=f"bnm{tag}")
        var = spool.tile([8, w], f32, name="bnvar", tag=f"bnv{tag}")
        t = spool.tile([8, w], f32, name="bnt", tag=f"bnt{tag}")
        y = spool.tile([8, w], f32, name="bny", tag=f"bny{tag}")
        nc.vector.tensor_scalar_mul(mean[:], sums, 1.0 / n)
        nc.vector.tensor_scalar_mul(var[:], sqs, 1.0 / n)         # E[x^2]
        nc.vector.tensor_mul(t[:], mean[:], mean[:])
        nc.vector.tensor_sub(var[:], var[:], t[:])
        nc.vector.tensor_scalar_add(var[:], var[:], EPS)
        # y0 = bits(0x5f3759df - (bits(var) >> 1)); two Newton steps
        nc.vector.tensor_scalar(y[:].bitcast(i32), var[:].bitcast(i32),
                                1, None, OP.arith_shift_right)
        nc.vector.scalar_tensor_tensor(y[:].bitcast(i32),
                                       magic[0:8, 0:w].bitcast(i32), 1,
                                       y[:].bitcast(i32), OP.mult,
                                       OP.subtract)
        for _ in range(2):
            nc.vector.tensor_mul(t[:], var[:], y[:])
            nc.vector.tensor_mul(t[:], t[:], y[:])
            nc.vector.tensor_scalar(t[:], t[:], -0.5, 1.5, OP.mult, OP.add)
            nc.vector.tensor_mul(y[:], y[:], t[:])
        nc.vector.tensor_mul(scales, y[:], gammas)
        nc.vector.tensor_mul(t[:], mean[:], scales)
        nc.vector.tensor_sub(shifts, betas, t[:])

    def broadcast128(src_sb, width, tag):
        """Replicate an [8, width] SBUF tile to [128, width] via DRAM,
        using a zero-stride source access pattern (one DMA)."""
        dd = dpool.tile([8, width], f32, name=f"bd{tag}", tag=f"bd{tag}")
        nc.sync.dma_start(dd[:], src_sb[:])
        bc = spool.tile([128, width], f32, name=f"bc{tag}", tag=f"bc{tag}")
        src = dd[:]
        rep = dataclasses.replace(src, ap=[[0, 16]] + list(src.ap))
        nc.sync.dma_start(bc[:], rep)
        return bc

    allst = allreduce_stats(st4[:, 0:4], 4, "hc")
    sb4 = spool.tile([8, 4], f32, name="sb4")
    # cols: scale_h, scale_c, shift_h, shift_c
    bn_coeffs(allst[:, 0:2], allst[:, 2:4], float(B), bn6[0:8, 0:2],
              bn6[0:8, 3:5], sb4[:, 0:2], sb4[:, 2:4], 2, "hc")
    bc4 = broadcast128(sb4, 4, "hc")
    nc.scalar.activation(H[:].bitcast(f32r), H[:], AF.Identity,
                         bias=bc4[:, 2:3], scale=bc4[:, 0:1])
    nc.scalar.activation(C[:], C[:], AF.Identity,
                         bias=bc4[:, 3:4], scale=bc4[:, 1:2])

    # ---- phase 2: autoregressive LSTM (relu), 14 steps ------------------
    # The bars batch-norm statistics are accumulated per step into spare
    # ScalarE capacity here, so the sync-2 bubble skips the big reduction.
    hs = state.tile([128, ROWS * J], f32, name="hs")
    scr3 = apool.tile([128, J], f32, name="scr3", tag="scr")
    stp2 = spool.tile([128, 2 * ROWS], f32, name="stp2")
    hin = H[:]
    for r in range(ROWS):
        z = zpool.tile([128, 4 * J], f32, name="z2", tag="z")
        for g in (2, 0, 1, 3):
            nc.tensor.matmul(z[:, g * J:(g + 1) * J], r_(u12[g][:]),
                             r_(hin), start=True, stop=True)
        hout = hs[:, r * J:(r + 1) * J]
        gates_elemwise(z, C, hout, relu_mode=True,
                       biases=None if zero_bias else b1v, np_=128,
                       relu_bias=None if zero_bias else b1v[2])
        nc.scalar.activation(scr3[:], hout, AF.Identity,
                             accum_out=stp2[:, 2 * r:2 * r + 1])
        nc.scalar.activation(scr3[:], hout, AF.Square,
                             accum_out=stp2[:, 2 * r + 1:2 * r + 2])
        hin = hout

    # ---- batch-norm stats for bars + AllReduce --------------------------
    st2 = spool.tile([128, 2], f32, name="st2")
    nc.vector.tensor_reduce(st2[:, 0:1], stp2.rearrange("p (r c) -> p r c",
                                                        c=2)[:, :, 0],
                            op=OP.add, axis=mybir.AxisListType.X)
    nc.vector.tensor_reduce(st2[:, 1:2], stp2.rearrange("p (r c) -> p r c",
                                                        c=2)[:, :, 1],
                            op=OP.add, axis=mybir.AxisListType.X)
    allst3 = allreduce_stats(st2[:, 0:2], 2, "b3")
    sb2 = spool.tile([8, 2], f32, name="sb2")
    bn_coeffs(allst3[:, 0:1], allst3[:, 1:2], float(B * ROWS),
              bn6[0:8, 2:3], bn6[0:8, 5:6], sb2[:, 0:1], sb2[:, 1:2],
              1, "b3")
    bc2 = broadcast128(sb2, 2, "b3")

    # Fold BN-3 into the backward-LSTM input weights/biases:
    #   (x*s + m) @ W2 = x @ (diag(s) W2) + (m @ W2)
    w2s = []
    for g in range(4):
        w = spool.tile([128, 64], f32r, name=f"w2s{g}", tag=f"w2s{g}")
        nc.vector.tensor_scalar_mul(w[:], w2b[g][:], bc2[:, 0:1])
        w2s.append(w)
    zb = zpool.tile([128, 4 * J], f32, name="zb", tag="z")
    for g in range(4):
        nc.tensor.matmul(zb[0:64, g:g + 1], w2b[g][:], bc2[:, 1:2],
                         start=True, stop=True)
    b3eff = []
    for g in range(4):
        bg = spool.tile([64, 1], f32, name=f"b3e{g}", tag=f"b3e{g}")
        nc.vector.scalar_tensor_tensor(bg[:], zb[0:64, g:g + 1], 1.0,
                                       b2v[g][:], OP.mult, OP.add)
        b3eff.append(bg)

    # ---- phase 3: backward LSTM (relu), reversed inputs -----------------
    # Same software pipeline: the input-side matmuls (which depend only on
    # hs) are issued one step ahead of the recurrent h-part.
    C3 = state.tile([64, J], f32, name="C3")
    nc.gpsimd.memset(C3[:], 0.0)

    def x3_step(r):
        src = hs[:, (ROWS - 1 - r) * J:(ROWS - r) * J]
        z = zpool.tile([128, 4 * J], f32, name="z3", tag="z")
        for g in range(4):
            nc.tensor.matmul(z[0:64, g * J:(g + 1) * J], r_(w2s[g][:]),
                             r_(src), start=True, stop=(r == 0),
                             skip_group_check=True)
        return z

    h3cur = None
    zc = x3_step(0)
    for r in range(ROWS):
        zn = x3_step(r + 1) if r + 1 < ROWS else None
        if r > 0:
            for g in (2, 0, 1, 3):
                nc.tensor.matmul(zc[0:64, g * J:(g + 1) * J], r_(u2b[g][:]),
                                 r_(h3cur[:]), start=False, stop=True,
                                 skip_group_check=True)
        h3 = h3pool.tile([64, J], f32, name="h3", tag="h3")
        gates_elemwise(zc, C3, h3[:], relu_mode=True, biases=b3eff, np_=64,
                       relu_bias=b3eff[2])
        nc.sync.dma_start(a["out"][r], h3[:])
        h3cur = h3
        zc = zn


INPUT_SHAPES = {
    "x": ([T, 2, 128, J], "f32"), "wx": ([4, 2, 128, 128], "f32"),
    "uh": ([4, 128, 128], "f32"), "u12": ([4, 128, 128], "f32"),
    "w2b": ([4, 128, 64], "f32"), "u2b": ([4, 64, 64], "f32"),
    "bx": ([4, 128, 1], "f32"), "b1v": ([4, 128, 1], "f32"),
    "b2v": ([4, 64, 1], "f32"), "bn6": ([8, 6], "f32"),
    "emat": ([128, 8], "f32"), "magic": ([8, 2], "i32"),
}


def _build_program(zero_bias=True):
    import concourse.bacc as bacc
    import concourse.tile as tile
    import concourse.mybir as mybir
    from contextlib import ExitStack

    dts = {"f32": mybir.dt.float32, "i32": mybir.dt.int32}
    nc = bacc.Bacc("TRN2", target_bir_lowering=False, debug=False,
                   enable_asserts=True, num_devices=NCORES)
    aps = {}
    for name, (shp, dt) in INPUT_SHAPES.items():
        aps[name] = nc.dram_tensor(name, shp, dts[dt],
                                   kind="ExternalInput")[:]
    aps["out"] = nc.dram_tensor("out", [ROWS, 64, J], mybir.dt.float32,
                                kind="ExternalOutput")[:]
    with tile.TileContext(nc) as tc:
        with ExitStack() as ctx:
            build_body(ctx, tc, aps, zero_bias=zero_bias)
    nc.compile()
    return nc, list(INPUT_SHAPES.keys())


def _get_program(zero_bias=True):
    global _PROG
    if not isinstance(_PROG, dict):
        _PROG = {}
    if zero_bias not in _PROG:
        _PROG[zero_bias] = _build_program(zero_bias)
    return _PROG[zero_bias]


# ---------------------------------------------------------------------------
# host-side data prep
# ---------------------------------------------------------------------------

def prep_shared(W0, U0, b0, gamma_h, beta_h, gamma_c, beta_c,
                W1, U1, b1, gamma3, beta3, W2, U2, b2):
    f = np.float32
    W0, U0, b0 = np.asarray(W0, f), np.asarray(U0, f), np.asarray(b0, f)
    W1, U1, b1 = np.asarray(W1, f), np.asarray(U1, f), np.asarray(b1, f)
    W2, U2, b2 = np.asarray(W2, f), np.asarray(U2, f), np.asarray(b2, f)
    U12 = W1 + U1

    wx = np.zeros((4, 2, 128, 128), f)
    uh = np.zeros((4, 128, 128), f)
    u12 = np.zeros((4, 128, 128), f)
    w2b = np.zeros((4, 128, 64), f)
    u2b = np.zeros((4, 64, 64), f)
    for g in range(4):
        for k in range(8):
            blk = W0[:, g * D:(g + 1) * D]
            wx[g, 0, 16 * k:16 * k + 16, 8 * k:8 * k + 8] = blk
            wx[g, 1, 16 * k:16 * k + 16, 64 + 8 * k:64 + 8 * k + 8] = blk
        for k in range(G):
            uh[g, 8 * k:8 * k + 8, 8 * k:8 * k + 8] = U0[:, g * D:(g + 1) * D]
            u12[g, 8 * k:8 * k + 8, 8 * k:8 * k + 8] = U12[:, g * D:(g + 1) * D]
            w2b[g, 8 * k:8 * k + 8, 4 * k:4 * k + 4] = W2[:, g * DO:(g + 1) * DO]
            u2b[g, 4 * k:4 * k + 4, 4 * k:4 * k + 4] = U2[:, g * DO:(g + 1) * DO]

    bx = np.stack([np.tile(b0[g * D:(g + 1) * D], G) for g in range(4)])
    b1v = np.stack([np.tile(b1[g * D:(g + 1) * D], G) for g in range(4)])
    b2v = np.stack([np.tile(b2[g * DO:(g + 1) * DO], G) for g in range(4)])
    bn6 = np.stack([np.asarray(v, f) for v in
                    (gamma_h, gamma_c, gamma3, beta_h, beta_c, beta3)],
                   axis=1)                          # [8, 6]
    emat = np.zeros((128, 8), f)
    emat[np.arange(128), np.arange(128) % 8] = 1.0
    magic = np.full((8, 2), 0x5F3759DF, np.int32)

    return {
        "wx": wx, "uh": uh, "u12": u12, "w2b": w2b, "u2b": u2b,
        "bx": bx[:, :, None].astype(f), "b1v": b1v[:, :, None].astype(f),
        "b2v": b2v[:, :, None].astype(f), "bn6": np.ascontiguousarray(bn6),
        "emat": emat, "magic": magic,
    }


def prep_x(noise_seed):
    """[B, T, F] -> per-core [T, 2, 128, J]; partition = 8 groups x 16
    features, the two halves covering groups 0-7 / 8-15."""
    x = np.asarray(noise_seed, np.float32)
    xr = x.reshape(NCORES, G, J, T, F).transpose(0, 3, 1, 4, 2)
    return np.ascontiguousarray(xr.reshape(NCORES, T, 2, 128, J))


def assemble_out(per_core):
    """8 x [ROWS, 64, J] device tensors -> [B, ROWS, DO]."""
    dev = np.stack(per_core)                     # [8, ROWS, 64, J]
    dev = dev.reshape(NCORES, ROWS, G, DO, J).transpose(0, 2, 4, 1, 3)
    return np.ascontiguousarray(dev.reshape(B, ROWS, DO)).astype(np.float32)


def make_in_maps(noise_seed, shared):
    xs = prep_x(noise_seed)
    return [{"x": xs[c], **shared} for c in range(NCORES)]


# ---------------------------------------------------------------------------
# entry point
# ---------------------------------------------------------------------------

def kernel(noise_seed, W0, U0, b0, gamma_h, beta_h, gamma_c, beta_c,
           W1, U1, b1, gamma3, beta3, W2, U2, b2, training=None, **_):
    from concourse import bass_utils

    zb = not (np.asarray(b0).any() or np.asarray(b1).any()
              or np.asarray(b2).any())
    nc, _names = _get_program(zero_bias=zb)
    shared = prep_shared(W0, U0, b0, gamma_h, beta_h, gamma_c, beta_c,
                         W1, U1, b1, gamma3, beta3, W2, U2, b2)
    in_maps = make_in_maps(noise_seed, shared)
    res = bass_utils.run_bass_kernel_spmd(
        nc, in_maps, core_ids=list(range(NCORES)),
        trace=bool(int(os.environ.get("KERNEL_TRACE", "0"))),
        tmpdir=os.environ.get("KERNEL_TRACE_DIR") or None)
    outs = [res.results[c]["out"] for c in range(NCORES)]
    out = assemble_out(outs)
    if res.exec_time_ns is not None:
        kernel.last_exec_time_ns = res.exec_time_ns
    return out
